# revision 5
# baseline (speedup 1.0000x reference)
"""Trainium2 Bass kernel v4 — folded-Wqk fp8 attention + folded PV + bf16 FFN.

Reference (per batch b):
    xn = LN(x[b]); yn = LN(y[b])
    q = xn@Wq; k = yn@Wk; v = yn@Wv
    a = softmax(mask(q@k^T/sqrt(L)))
    x2 = xn + a@v; x3 = LN(x2)
    out1 = x3 + relu(x3@Win)@Wout
    returns (out1, yn)

Algebraic folds:
  1. q@k^T = xn @ (Wq@Wk^T) @ yn^T   (Wqk on host)
  2. a@v   = (a@yn) @ Wv             (PV fold: drops the yn@Wv GEMM)

Sharding: 8 cores = 4 batches x 2 query-halves; yn duplicated across the
pair; everything SBUF-resident.

fp8 scheme (e4m3, max 448):
    xn8/yn8 = 16*LN fp8; Wqk8 = 256*Wqk, Wv8 = 16*Wv (host)
    s1T8 = psum/512 = 8*s1
    logits = s1T8.yn8/(128*64); STexp = 16*exp(logits-5)*keep01 fp8
    UT8 = (STexp@yn8)/32 = 8*(w@yn); Z via ones-cols chunk of yn8,
    transposed to q-partitions; x2 = UT8@Wv8 * 1/(128*Z_w) + xn
    FFN bf16.
"""

import numpy as np
import sys

for _p in ("/opt/trn_rl_repo",):
    if _p not in sys.path:
        sys.path.insert(0, _p)

import concourse.bass as bass
import concourse.bacc as bacc
import concourse.mybir as mybir
import concourse.tile as tile
from concourse.bass_utils import run_bass_kernel_spmd
from concourse.masks import make_identity

import ml_dtypes

P = 128
E = 1024          # embedding dim
L = 4096          # latent dim
SK = 2048         # key rows per batch
SQH = 1024        # query rows per core (half batch)
B = 4
NCORES = 8
EC = E // P       # 8
LC = L // P       # 32
KC = SK // P      # 16
QT = SQH // P     # 8
EW = E + P        # yn8 width: E cols + [8 ones | 120 zero] Z-chunk
EXP_SHIFT = 5.0
EXP_SCALE = 1.0 / (128.0 * 64.0)

F32 = mybir.dt.float32
BF16 = mybir.dt.bfloat16
F8 = mybir.dt.float8e4
NP8 = ml_dtypes.float8_e4m3
NPB = ml_dtypes.bfloat16

AF = mybir.ActivationFunctionType
OP = mybir.AluOpType
DR = mybir.MatmulPerfMode.DoubleRow

_CACHE = {}


def _layernorm_tile(nc, pool, out_ap, in_ap, eps_tile, eps256_tile=None,
                    out8_ap=None):
    """LN over the free dim (1024) of a [128, 1024] f32 tile.
    If out8_ap given, also writes 16*LN(x) (fp8) via a second apply using
    rstd/16 computed as rsqrt((var+eps)/256) -- no extra Act work."""
    stats = pool.tile([P, 2, 6], F32, tag="ln_stats")
    mv = pool.tile([P, 2], F32, tag="ln_mv")
    xr = in_ap.rearrange("p (s d) -> p s d", s=2)
    for s in range(2):
        nc.vector.bn_stats(out=stats[:, s, :], in_=xr[:, s, :])
    nc.vector.bn_aggr(out=mv[:], in_=stats[:])
    sd = pool.tile([P, 1], F32, tag="ln_sd")
    nc.scalar.activation(out=sd[:], in_=mv[:, 1:2], func=AF.Sqrt, bias=eps_tile[:])
    rs = pool.tile([P, 1], F32, tag="ln_rs")
    nc.vector.reciprocal(out=rs[:], in_=sd[:])
    if out8_ap is not None:
        # swap engines: bf16 main output on Pool, fp8 on DVE -- DVE is the
        # phase-1 bottleneck and fp8 output is its cheaper variant
        nc.gpsimd.tensor_scalar(
            out=out_ap, in0=in_ap, scalar1=mv[:, 0:1], scalar2=rs[:],
            op0=OP.subtract, op1=OP.mult,
        )
        sd16 = pool.tile([P, 1], F32, tag="ln_sd16")
        nc.scalar.activation(out=sd16[:], in_=mv[:, 1:2], func=AF.Sqrt,
                             bias=eps256_tile[:], scale=1.0 / 256.0)
        rs16 = pool.tile([P, 1], F32, tag="ln_rs16")
        nc.vector.reciprocal(out=rs16[:], in_=sd16[:])
        nc.vector.tensor_scalar(
            out=out8_ap, in0=in_ap, scalar1=mv[:, 0:1], scalar2=rs16[:],
            op0=OP.subtract, op1=OP.mult,
        )
    else:
        nc.vector.tensor_scalar(
            out=out_ap, in0=in_ap, scalar1=mv[:, 0:1], scalar2=rs[:],
            op0=OP.subtract, op1=OP.mult,
        )


def _build(sim=False, phases="12spf", reps=1):
    nc = bacc.Bacc("TRN2", target_bir_lowering=False, debug=False,
                   num_devices=1 if sim else NCORES)

    x_h = nc.dram_tensor("x_h", [SQH, E], F32, kind="ExternalInput")
    y_b = nc.dram_tensor("y_b", [SK, E], F32, kind="ExternalInput")
    keepT = nc.dram_tensor("keepT", [KC, P, SQH], F8, kind="ExternalInput")
    Wqk8 = nc.dram_tensor("Wqk8", [P, EC, EC, P], F8, kind="ExternalInput")
    Wv8 = nc.dram_tensor("Wv8", [P, EC, E], F8, kind="ExternalInput")
    WinB = nc.dram_tensor("WinB", [LC, P, EC, P], BF16, kind="ExternalInput")
    WoutB = nc.dram_tensor("WoutB", [P, LC, E], BF16, kind="ExternalInput")

    out1 = nc.dram_tensor("out1", [SQH, E], BF16, kind="ExternalOutput")
    yn_out = nc.dram_tensor("yn_out", [SK, E], BF16, kind="ExternalOutput")

    with tile.TileContext(nc) as tc:
        for _ in range(reps):
            _graph(nc, tc, x_h, y_b, keepT, Wqk8, Wv8, WinB, WoutB,
                   out1, yn_out, phases)
    nc.compile()
    return nc


def _graph(nc, tc, x_h, y_b, keepT, Wqk8, Wv8, WinB, WoutB,
           out1, yn_out, phases="12spf"):
    with tc.tile_pool(name="consts", bufs=1) as consts:
        ident8 = consts.tile([P, P], F8)
        make_identity(nc, ident8[:])
        identb = consts.tile([P, P], BF16)
        make_identity(nc, identb[:])
        identf = consts.tile([P, P], F32)
        make_identity(nc, identf[:])
        eps_t = consts.tile([P, 1], F32)
        nc.vector.memset(eps_t[:], 1e-5)
        eps256_t = consts.tile([P, 1], F32)
        nc.vector.memset(eps256_t[:], 1e-5 / 256.0)
        ebias = consts.tile([P, 1], F32)
        nc.vector.memset(ebias[:], float(-EXP_SHIFT + np.log(16.0)))

        with tc.tile_pool(name="p_x3", bufs=1) as p_x3:
            x3T = p_x3.tile([P, EC, SQH], BF16)     # 2 MB
            with tc.tile_pool(name="p_attn", bufs=1) as p_attn:
                xn = p_attn.tile([P, QT, E], BF16)      # 2 MB
                x3b = xn  # x3 overwrites xn in place (xn[qt] dies at the
                          # stage-2 fused add; LN then reclaims the rows)
                yn8 = p_attn.tile([P, KC, EW], F8)      # 2.25 MB
                STexp = p_attn.tile([P, KC, SQH], F8)   # 2 MB
                UT8 = p_attn.tile([P, EC, SQH], F8)     # 1 MB
                wv = p_attn.tile([P, EC, E], F8)        # 1 MB
                with tc.tile_pool(name="p_proj", bufs=1) as p_proj:
                    ynT = p_proj.tile([P, EC, SK], F8)      # 2 MB
                    xnT = p_proj.tile([P, EC, SQH], F8)     # 1 MB
                    s1T = p_proj.tile([P, EC, SQH], F8)     # 1 MB
                    wqk = p_proj.tile([P, EC, EC, P], F8)   # 1 MB
                    keep_s = p_proj.tile([P, KC, SQH], F8)  # 2 MB
                    _phase_1(nc, tc, x_h, y_b, yn_out,
                             xn, yn8, ynT, xnT, ident8, eps_t, eps256_t,
                             wqk, Wqk8, s1T, keep_s, keepT, wv, Wv8,
                             STexp, ebias, phases)
                if "p" in phases:
                    _phase_pv(nc, tc, xn, yn8, wv, STexp, UT8,
                              x3b, x3T, identb, identf, eps_t,
                              WinB, WoutB, out1, do_ffn=("f" in phases))


def _ln_row_tile(nc, p_in, p_tmp, p_n8, p_tp, ident8, eps_t, eps256_t,
                 src_t, row0, nT, spill_dst, keep_dst, t8_dst=None):
    """LN one [128, E] row tile; dual-output (f32/bf16 + 16x fp8) on DVE,
    8 PE transposes batched into one PSUM bank, single copy into nT."""
    t_in = p_in.tile([P, E], F32, tag="ln_in")
    nc.sync.dma_start(out=t_in[:], in_=src_t[row0:row0 + P, :])
    if keep_dst is None:
        t_n = p_in.tile([P, E], BF16, tag="ln_out")
        dst = t_n[:]
    else:
        dst = keep_dst
    if t8_dst is None:
        t8_t = p_n8.tile([P, E], F8, tag="ln8")
        t8 = t8_t[:]
    else:
        t8 = t8_dst
    _layernorm_tile(nc, p_tmp, dst, t_in[:], eps_t, eps256_t, t8)
    if spill_dst is not None:
        nc.gpsimd.dma_start(out=spill_dst[row0:row0 + P, :], in_=dst)
    # 8 fp8 transposes into one 2KB PSUM bank (element step 2), one copy out
    ps = p_tp.tile([P, EC, P, 2], F8, tag="tp8")
    for ec in range(EC):
        nc.tensor.transpose(ps[:, ec, :, 0], t8[:, ec * P:(ec + 1) * P],
                            ident8[:])
    nc.scalar.copy(out=nT[:, :, row0:row0 + P], in_=ps[:, :, :, 0])


def _phase_1(nc, tc, x_h, y_b, yn_out,
             xn, yn8, ynT, xnT, ident8, eps_t, eps256_t,
             wqk, Wqk8, s1T, keep_s, keepT, wv, Wv8,
             STexp, ebias, phases="12spf"):
    with tc.tile_pool(name="p1_in", bufs=6) as p_in, \
         tc.tile_pool(name="p1_tmp", bufs=12) as p_tmp, \
         tc.tile_pool(name="p1_n8", bufs=4) as p_n8, \
         tc.tile_pool(name="p1_tp", bufs=2, space="PSUM") as p_tp, \
         tc.tile_pool(name="p1_mm", bufs=3, space="PSUM") as p_mm:

        def y_tile(t):
            _ln_row_tile(nc, p_in, p_tmp, p_n8, p_tp, ident8, eps_t,
                         eps256_t, y_b, t * P, ynT, yn_out, None,
                         t8_dst=yn8[:, t, 0:E])

        def x_tile(t):
            _ln_row_tile(nc, p_in, p_tmp, p_n8, p_tp, ident8, eps_t,
                         eps256_t, x_h, t * P, xnT, None, xn[:, t, :])
            if "2" in phases and t in (3, 7):
                qb = 0 if t == 3 else 1
                for ep in range(4):
                    ps = p_mm.tile([P, 2, 512], F32, tag="mm")
                    for h2 in range(2):
                        et = 2 * ep + h2
                        for c in range(4):
                            nc.tensor.matmul(
                                ps[:, h2, :], wqk[:, 2 * c:2 * c + 2, et, :],
                                xnT[:, 2 * c:2 * c + 2,
                                    qb * 512:(qb + 1) * 512],
                                start=(c == 0), stop=(c == 3), perf_mode=DR)
                    if ep % 2 == 0:
                        nc.vector.tensor_scalar_mul(
                            out=s1T[:, 2 * ep:2 * ep + 2,
                                    qb * 512:(qb + 1) * 512],
                            in0=ps[:], scalar1=1.0 / 512.0)
                    else:
                        nc.scalar.activation(
                            out=s1T[:, 2 * ep:2 * ep + 2,
                                    qb * 512:(qb + 1) * 512],
                            in_=ps[:], func=AF.Copy, scale=1.0 / 512.0)

        def score(kc):
            """ST[k,q] = exp(ynT.s1T scaled - shift) * keep, fp8."""
            ps = p_mm.tile([P, 2, 512], F32, tag="mm")
            for qc in range(2):
                for c in range(4):
                    nc.tensor.matmul(
                        ps[:, qc, :],
                        ynT[:, 2 * c:2 * c + 2, kc * P:(kc + 1) * P],
                        s1T[:, 2 * c:2 * c + 2, qc * 512:(qc + 1) * 512],
                        start=(c == 0), stop=(c == 3), perf_mode=DR)
            nc.scalar.activation(
                out=STexp[:, kc, :], in_=ps[:].rearrange("p a b -> p (a b)"),
                func=AF.Exp, bias=ebias[:], scale=EXP_SCALE)
            nc.vector.tensor_tensor(
                out=STexp[:, kc, :], in0=STexp[:, kc, :],
                in1=keep_s[:, kc, :], op=OP.mult)

        # x tiles first (s1 after qb halves); weight/keep DMAs staggered
        # between tile loads so the DMA bus never blocks the LN pipeline.
        x_tile(0)
        nc.sync.dma_start(out=wqk[:], in_=Wqk8.ap())
        for t in range(1, QT):
            x_tile(t)
            if t < 5:
                # keep chunks land on the scalar HWDGE queue, 4 tiles ahead
                for kc in range(4 * (t - 1), 4 * t):
                    nc.scalar.dma_start(out=keep_s[:, kc, :],
                                        in_=keepT.ap()[kc])
        # y tiles in blocks of 4; block b's scores run after block b+1's
        # LNs so the Act engine alternates sqrt-table and exp-table work
        # once per block (not per tile) -- each switch costs a 1.28us
        # activation-table reload on HW
        for t in range(KC):
            y_tile(t)
            if t == 0:
                nc.scalar.dma_start(out=wv[:], in_=Wv8.ap())
            if "s" in phases and t % 4 == 3 and t >= 7:
                for kc in range(t - 7, t - 3):
                    score(kc)
        if "s" in phases:
            for kc in range(KC - 4, KC):
                score(kc)


def _phase_pv(nc, tc, xn, yn8, wv, STexp, UT8, x3b, x3T, identb, identf,
              eps_t, WinB=None, WoutB=None, out1=None, do_ffn=False):
    """UT8 = (STexp@yn8)/32 fp8 [e-part, q]; Z-chunk -> rec[q] via PE
    transpose; x2 = UT8@wv * rec + xn (fused); x3 = LN(x2); FFN interleaved
    with the second half of stage 2 so the PE never drains."""
    # Z-chunk of yn8: cols [E:E+8] = 1, [E+8:E+128] = 0 (placed here so the
    # memsets never delay the phase-1 LN pipeline)
    nc.vector.memset(yn8[:, :, E:EW], 0.0)
    nc.vector.memset(yn8[:, :, E:E + 8], 1.0)
    with tc.tile_pool(name="pv_x", bufs=2) as p_x, \
         tc.tile_pool(name="pv_tmp", bufs=10) as p_tmp, \
         tc.tile_pool(name="pv_mm", bufs=2, space="PSUM") as p_mm, \
         tc.tile_pool(name="pv_tp", bufs=2, space="PSUM") as p_tp, \
         tc.tile_pool(name="pf_h", bufs=1) as p_h, \
         tc.tile_pool(name="pf_w", bufs=4) as p_w, \
         tc.tile_pool(name="pf_wo", bufs=2) as p_wo, \
         tc.tile_pool(name="pf_o", bufs=2) as p_o:

        # ---- stage 1: Z rows first, then UT8[e, q] chunks; then Z rows
        # -> q-partition layout via PE transpose; rec = 1/(128 Z_w) ----
        recs = []
        with tc.tile_pool(name="pv_z", bufs=1) as p_z:
            zrow = p_z.tile([P, 2, 512], F32, tag="zrow")
            for ec in [EC] + list(range(EC)):
                ps = p_mm.tile([P, 2, 512], F32, tag="pv")
                for qh in range(2):
                    for j in range(KC // 2):
                        nc.tensor.matmul(
                            ps[:, qh, :],
                            yn8[:, 2 * j:2 * j + 2, ec * P:(ec + 1) * P],
                            STexp[:, 2 * j:2 * j + 2,
                                  qh * 512:(qh + 1) * 512],
                            start=(j == 0), stop=(j == KC // 2 - 1),
                            perf_mode=DR)
                if ec < EC:
                    nc.scalar.activation(
                        out=UT8[:, ec, :],
                        in_=ps[:].rearrange("p a b -> p (a b)"),
                        func=AF.Copy, scale=1.0 / 32.0)
                else:
                    nc.scalar.copy(out=zrow[:], in_=ps[:])

            with tc.tile_pool(name="pv_zt", bufs=1, space="PSUM") as p_zt:
                zt = p_zt.tile([P, QT, P], F32, tag="zt")
                for qt in range(QT):
                    nc.tensor.transpose(
                        zt[:, qt, :],
                        zrow[:, qt // 4, (qt % 4) * P:(qt % 4 + 1) * P],
                        identf[:])
                    z8 = p_tmp.tile([P, 1], F32, tag="z8")
                    nc.vector.tensor_scalar_mul(
                        out=z8[:], in0=zt[:, qt, 0:1], scalar1=8.0)
                    rec = p_tmp.tile([P, 1], F32, tag="rec")
                    nc.vector.reciprocal(out=rec[:], in_=z8[:])
                    recs.append(rec)

        # ---- stage 2 (x2 -> x3), interleaved with FFN ----
        def stage2_gemm(qt):
            qsl = slice(qt * P, (qt + 1) * P)
            ps2 = p_mm.tile([P, 2, 512], F32, tag="pv")
            for eh in range(2):
                for c in range(4):
                    nc.tensor.matmul(
                        ps2[:, eh, :],
                        UT8[:, 2 * c:2 * c + 2, qsl],
                        wv[:, 2 * c:2 * c + 2, eh * 512:(eh + 1) * 512],
                        start=(c == 0), stop=(c == 3), perf_mode=DR)
            x2 = p_x.tile([P, E], BF16, tag="x2")
            nc.vector.scalar_tensor_tensor(
                out=x2[:], in0=ps2[:].rearrange("p a b -> p (a b)"),
                scalar=recs[qt][:],
                in1=xn[:, qt, :], op0=OP.mult, op1=OP.add)
            _layernorm_tile(nc, p_tmp, x3b[:, qt, :], x2[:], eps_t)

        def stage2_tp(qt):
            qsl = slice(qt * P, (qt + 1) * P)
            pst = p_tp.tile([P, EC, P], BF16, tag="tpb")
            for ec in range(EC):
                nc.tensor.transpose(
                    pst[:, ec, :], x3b[:, qt, ec * P:(ec + 1) * P], identb[:])
            nc.scalar.copy(out=x3T[:, :, qsl], in_=pst[:])

        if not do_ffn:
            for qt in range(QT):
                stage2_gemm(qt)
                stage2_tp(qt)
            return


        wos = []

        def ffn1(qb):
            hT = p_h.tile([P, LC, 512], BF16, tag="hT")   # 4 MB
            wis = []

            def wi_load(lt):
                wi_t = p_w.tile([P, EC, P], BF16, tag="wi")
                nc.sync.dma_start(out=wi_t[:], in_=WinB.ap()[lt])
                wis.append(wi_t)

            for lt in range(3):
                wi_load(lt)
            with tc.tile_pool(name="pf_mm", bufs=2, space="PSUM") as p_mm1:
                for lt in range(LC):
                    if qb == 0 and lt in (8, 24):
                        eh = 0 if lt == 8 else 1
                        wo = p_wo.tile([P, LC, 512], BF16, tag="wo")
                        nc.scalar.dma_start(
                            out=wo[:],
                            in_=WoutB.ap()[:, :, eh * 512:(eh + 1) * 512])
                        wos.append(wo)
                    if lt + 3 < LC:
                        wi_load(lt + 3)
                    wi_t = wis[lt]
                    ps = p_mm1.tile([P, 512], F32, tag="h")
                    for ec in range(EC):
                        nc.tensor.matmul(
                            ps[:], wi_t[:, ec, :],
                            x3T[:, ec, qb * 512:(qb + 1) * 512],
                            start=(ec == 0), stop=(ec == EC - 1))
                    nc.scalar.activation(
                        out=hT[:, lt, :], in_=ps[:], func=AF.Relu)
            return hT

        def ffn2(qts, hT):
            with tc.tile_pool(name="pf_mm2", bufs=2, space="PSUM") as p_mm2:
                for qt in qts:
                    qsl = slice(qt * P, (qt + 1) * P)
                    for eh in range(2):
                        esl = slice(eh * 512, (eh + 1) * 512)
                        ps = p_mm2.tile([P, 512], F32, tag="f")
                        for lc in range(LC):
                            nc.tensor.matmul(
                                ps[:], hT[:, lc, qt % 4 * P:(qt % 4 + 1) * P],
                                wos[eh][:, lc, :],
                                start=(lc == 0), stop=(lc == LC - 1))
                        o_t = p_o.tile([P, 512], BF16, tag="o")
                        nc.vector.tensor_add(
                            out=o_t[:], in0=ps[:], in1=x3b[:, qt, esl])
                        nc.sync.dma_start(
                            out=out1.ap()[qsl, esl], in_=o_t[:])

        for qt in range(4):
            stage2_gemm(qt)
        for qt in range(4):
            stage2_tp(qt)
        hT0 = ffn1(0)
        for qt in range(4, QT):
            stage2_gemm(qt)
        for qt in range(4, QT):
            stage2_tp(qt)
        ffn2(range(0, 4), hT0)
        hT1 = ffn1(1)
        ffn2(range(4, QT), hT1)


def _get_compiled(sim=False, phases="12spf"):
    key = (sim, phases)
    if key not in _CACHE:
        _CACHE[key] = _build(sim, phases)
    return _CACHE[key]


def _check_trivial(inputs):
    for n in ("ln1_w", "ln2_w", "ln3_w"):
        if n in inputs and not np.allclose(np.asarray(inputs[n]), 1.0):
            raise NotImplementedError(f"nontrivial {n} unsupported")
    for n in ("ln1_b", "ln2_b", "ln3_b", "bq", "bk", "bv", "bin", "bout"):
        if n in inputs and not np.allclose(np.asarray(inputs[n]), 0.0):
            raise NotImplementedError(f"nontrivial {n} unsupported")


def _prep_weights(inputs):
    Wq = np.asarray(inputs["Wq"], np.float32)
    Wk = np.asarray(inputs["Wk"], np.float32)
    Wv = np.asarray(inputs["Wv"], np.float32)
    Win = np.asarray(inputs["Win"], np.float32)
    Wout = np.asarray(inputs["Wout"], np.float32)
    Wqk = Wq @ Wk.T               # [E, E] folded scores weight
    def wtile(w, cdim):
        return np.ascontiguousarray(
            w.reshape(EC, P, cdim, P).transpose(2, 1, 0, 3))
    return {
        "Wqk8": np.ascontiguousarray(
            (256.0 * Wqk).reshape(EC, P, EC, P).transpose(1, 0, 2, 3)
        ).astype(NP8),
        "Wv8": np.ascontiguousarray(
            (16.0 * Wv).reshape(EC, P, E).transpose(1, 0, 2)).astype(NP8),
        "WinB": wtile(Win, LC).astype(NPB),
        "WoutB": np.ascontiguousarray(
            Wout.reshape(LC, P, E).transpose(1, 0, 2)).astype(NPB),
    }


LAST_EXEC_NS = None
TRACE = False


def kernel(**inputs):
    global LAST_EXEC_NS
    _check_trivial(inputs)
    x = np.ascontiguousarray(np.asarray(inputs["x"], dtype=np.float32))
    y = np.ascontiguousarray(np.asarray(inputs["y"], dtype=np.float32))
    mask = np.asarray(inputs["mask"])
    W = _prep_weights(inputs)

    nc = _get_compiled()
    in_maps = []
    for c in range(NCORES):
        b, h = c // 2, c % 2
        keep = (mask[b, h * SQH:(h + 1) * SQH] == 0).astype(np.float32)
        keepT_ = np.ascontiguousarray(keep.T.reshape(KC, P, SQH)).astype(NP8)
        in_maps.append({
            "x_h": np.ascontiguousarray(x[b, h * SQH:(h + 1) * SQH]),
            "y_b": y[b],
            "keepT": keepT_,
            **W,
        })
    last_err = None
    for attempt in range(3):
        try:
            res = run_bass_kernel_spmd(nc, in_maps,
                                       core_ids=list(range(NCORES)),
                                       trace=TRACE)
            break
        except Exception as e:   # transient device/terminal errors
            last_err = e
            import time as _time
            _time.sleep(10)
    else:
        raise last_err
    LAST_EXEC_NS = res.exec_time_ns
    outs = res.results
    o1 = np.empty((B, 2 * SQH, E), np.float32)
    yn = np.empty((B, SK, E), np.float32)
    for c in range(NCORES):
        b, h = c // 2, c % 2
        o1[b, h * SQH:(h + 1) * SQH] = outs[c]["out1"]
        if h == 0:
            yn[b] = outs[c]["yn_out"]
    return o1, yn
